# revision 1
# baseline (speedup 1.0000x reference)
"""Causal self-attention (B=4, T=2048, C=1024, NH=16) on 8 TRN2 NeuronCores.

Sharding (tensor-parallel heads x data-parallel batch):
  - 4 core-pairs: pair p = cores (2p, 2p+1) handles batch b = p.
  - Within a pair, rank 0 computes heads 0-7, rank 1 heads 8-15
    (w_qkv output columns split by head group).
  - After attention each core holds attnT [512, T] (d-major, its 8 heads).
    A pairwise AllGather exchanges these; each core then computes a
    512-column half of the output projection (w_proj column split), so no
    all-reduce is needed. Host concatenates the halves.

Device algorithm (per core):
  Phase 1  QKV: xT (c-major x, transposed on host) @ w slices on the PE.
           q/k are produced d-major ([c_out, t]); v t-major with a fused
           ones-column for softmax denominators. q is scaled by 1/8 and
           spilled to HBM (SBUF pressure), k/v stay resident.
  Phase 2  Attention per (head-pair j, 512-wide q block):
           sT[kv,q] = kT_chunk.T @ qT (K=64, two heads packed in PE row
           groups 0-63/64-127), exp on ScalarE (no max subtraction: scores
           are ~N(0,1) so exp cannot overflow), causal mask = one static
           128x128 triangle multiply on the diagonal chunk, then
           aug[65,q] += v_aug.T @ aT accumulated over kv chunks in PSUM.
           Row 64 of aug is the softmax denominator (ones column of v_aug);
           normalize via DVE reciprocal + GpSimd partition-broadcast + mul.
  Phase 3  Pairwise AllGather of attnT blocks (issued per head-pair so they
           overlap remaining attention), then y[t,512] = attnT.T @ w_proj
           half, accumulated over 8 c_in blocks of 128.

All matmuls run in float32r (TF32-like, full PE rate at free-dim >= 256,
measured ~1.5e-4 max rel err per matmul): every matmul-operand tile and its
DMA source is declared float32r (same bytes as fp32 host-side); PSUM stays
fp32.
"""

import numpy as np

import concourse.bass as bass
import concourse.mybir as mybir
import concourse.tile as tile
from concourse import bacc
from concourse.bass_utils import run_bass_kernel_spmd

B, T, C = 4, 2048, 1024
NH, HD = 16, 64
N_CORES = 8
HPC = NH // 2          # heads per core
NPAIR = HPC // 2       # head-pairs per core (PE row-group packing unit)
TB = T // 128          # 128-row t blocks
QBS = T // 512         # 512-wide q blocks
KC = C // 128          # 128-deep contraction chunks for qkv/proj
SCALE = float(1.0 / np.sqrt(HD))

F32 = mybir.dt.float32
F32R = mybir.dt.float32r
AF = mybir.ActivationFunctionType
REPLICA_GROUPS = [[0, 1], [2, 3], [4, 5], [6, 7]]


def build_nc(reps=1, single_core=False):
    nc = bacc.Bacc(
        "TRN2", target_bir_lowering=False, debug=False,
        num_devices=(1 if single_core else N_CORES),
    )

    xt = nc.dram_tensor("xt", [C, T], F32R, kind="ExternalInput")
    wq = nc.dram_tensor("wq", [C, 512], F32R, kind="ExternalInput")
    wk = nc.dram_tensor("wk", [C, 512], F32R, kind="ExternalInput")
    wv = nc.dram_tensor("wv", [C, 512], F32R, kind="ExternalInput")
    wp = nc.dram_tensor("wp", [C, 512], F32R, kind="ExternalInput")
    mask = nc.dram_tensor("mask", [128, 128], F32R, kind="ExternalInput")
    y = nc.dram_tensor("y", [T, 512], F32, kind="ExternalOutput")

    with tile.TileContext(nc) as tc:
        for _rep in range(reps):
            _emit_one(nc, tc, xt, wq, wk, wv, wp, mask, y, single_core)

    nc.compile()
    return nc


def _emit_one(nc, tc, xt, wq, wk, wv, wp, mask, y, single_core):
    with tc.tile_pool(name="qspilld", bufs=1, space="DRAM") as qspd, \
         tc.tile_pool(name="persist", bufs=1) as persist, \
         tc.tile_pool(name="ccin", bufs=NPAIR, space="DRAM") as ccinp, \
         tc.tile_pool(name="ccout", bufs=NPAIR, space="DRAM") as ccoutp, \
         tc.tile_pool(name="qt", bufs=2) as qtp, \
         tc.tile_pool(name="aT", bufs=3) as atp, \
         tc.tile_pool(name="norm", bufs=2) as normp, \
         tc.tile_pool(name="nb", bufs=2) as nbp, \
         tc.tile_pool(name="ps_s", bufs=2, space="PSUM") as pss, \
         tc.tile_pool(name="augA", bufs=2, space="PSUM") as augAp, \
         tc.tile_pool(name="augB", bufs=2, space="PSUM") as augBp:
        qspill = qspd.tile([NPAIR, 128, T], F32R)
        kT_sb = persist.tile([128, NPAIR, T], F32R)
        v_sb = persist.tile([128, TB, HPC, HD + 1], F32R)
        mask_sb = persist.tile([128, 128], F32R)
        nc.sync.dma_start(out=mask_sb[:], in_=mask[:])
        wp_sb = persist.tile([128, KC, 512], F32R)
        wpr = wp[:].rearrange("(a p) n -> p a n", p=128)

        cc_outs = []
        with tc.tile_pool(name="xtp", bufs=1) as xtp, \
             tc.tile_pool(name="wvp", bufs=1) as wvp, \
             tc.tile_pool(name="wqk", bufs=1) as wqkp, \
             tc.tile_pool(name="qsp", bufs=2) as qspp:
            ones_sc = qspp.tile([128, TB * HPC], F32, tag="ones")
            nc.vector.memset(ones_sc[:], 1.0)
            nc.vector.tensor_copy(
                v_sb[:, :, :, HD],
                ones_sc[:].rearrange("p (a b) -> p a b", a=TB),
            )
            xt_sb = xtp.tile([128, KC, T], F32R)
            xt_r = xt[:].rearrange("(a p) t -> p a t", p=128)
            wv_sb = wvp.tile([128, KC, 512], F32R)
            wv_r = wv[:].rearrange("(a p) n -> p a n", p=128)
            for a in range(KC):
                if a == 0:
                    nc.sync.dma_start(
                        out=xt_sb[:, 0, 0:1024], in_=xt_r[:, 0, 0:1024])
                    nc.sync.dma_start(out=wv_sb[:, 0, :], in_=wv_r[:, 0, :])
                    nc.sync.dma_start(
                        out=xt_sb[:, 0, 1024:T], in_=xt_r[:, 0, 1024:T])
                else:
                    nc.sync.dma_start(out=xt_sb[:, a, :], in_=xt_r[:, a, :])
                    nc.sync.dma_start(out=wv_sb[:, a, :], in_=wv_r[:, a, :])
            nc.sync.dma_start(out=wp_sb[:], in_=wpr[:])
            wq_r = wq[:].rearrange("(a p) n -> p a n", p=128)
            wk_r = wk[:].rearrange("(a p) n -> p a n", p=128)

            # v: out[t 128, c_out 512] = xT_chunk.T @ w_v  (t-block major)
            for tb in range(TB):
                ps = pss.tile([128, 1024], F32, tag="s2", name="psv")
                for a in range(KC):
                    nc.tensor.matmul(
                        ps[:, 0:512],
                        xt_sb[:, a, 128 * tb:128 * (tb + 1)],
                        wv_sb[:, a, :],
                        start=(a == 0), stop=(a == KC - 1),
                    )
                nc.vector.tensor_copy(
                    v_sb[:, tb, :, 0:HD],
                    ps[:, 0:512].rearrange("p (h d) -> p h d", h=HPC),
                )

            for j in range(NPAIR):
                # q(j), k(j): out[c_out 128, t 512] = w_block.T @ xT
                wqj = wqkp.tile([128, KC, 128], F32R, tag="wqj")
                wkj = wqkp.tile([128, KC, 128], F32R, tag="wkj")
                nc.sync.dma_start(
                    out=wqj[:], in_=wq_r[:, :, 128 * j:128 * (j + 1)])
                nc.sync.dma_start(
                    out=wkj[:], in_=wk_r[:, :, 128 * j:128 * (j + 1)])
                for which in ("q", "k"):
                    wsb = wqj if which == "q" else wkj
                    for i in range(QBS):
                        ps = pss.tile([128, 1024], F32, tag="s2", name="psqk")
                        for a in range(KC):
                            nc.tensor.matmul(
                                ps[:, 0:512],
                                wsb[:, a, :],
                                xt_sb[:, a, 512 * i:512 * (i + 1)],
                                start=(a == 0), stop=(a == KC - 1),
                            )
                        if which == "q":
                            sp = qspp.tile([128, 512], F32R, tag="qsp")
                            nc.vector.tensor_scalar_mul(
                                sp[:], ps[:, 0:512], SCALE)
                            nc.sync.dma_start(
                                out=qspill[j, :, 512 * i:512 * (i + 1)],
                                in_=sp[:],
                            )
                        else:
                            nc.vector.tensor_copy(
                                kT_sb[:, j, 512 * i:512 * (i + 1)],
                                ps[:, 0:512],
                            )

                # ---- attention for pair j (interleaves with next pair) ----
                ci = ccinp.tile([128, T], F32R, tag="ci", name="ci")
                for qb in range(QBS):
                    qt = qtp.tile([128, 512], F32R, tag="qt")
                    nc.sync.dma_start(
                        out=qt[:], in_=qspill[j, :, 512 * qb:512 * (qb + 1)]
                    )
                    nchunks = 4 * (qb + 1)
                    augs = [
                        augAp.tile([128, 512], F32, tag="augA", name="augA"),
                        augBp.tile([128, 512], F32, tag="augB", name="augB"),
                    ]
                    for c in range(nchunks):
                        diag = c >= 4 * qb
                        o = (c - 4 * qb) * 128 if diag else 0
                        mo = min(o, 256)  # keep matmul free dim >= 256
                        last = c == nchunks - 1
                        s2 = pss.tile([128, 1024], F32, tag="s2", name="s2")
                        for hh in range(2):
                            nc.tensor.matmul(
                                s2[:, 512 * hh + mo:512 * hh + 512],
                                kT_sb[64 * hh:64 * hh + 64, j,
                                      128 * c:128 * (c + 1)],
                                qt[64 * hh:64 * hh + 64, mo:512],
                                start=True, stop=True,
                            )
                        aT = atp.tile([128, 1024], F32R, tag="aT")
                        nc.scalar.activation(
                            aT[:].rearrange("p (h q) -> p h q", h=2)[:, :, o:512],
                            s2[:].rearrange("p (h q) -> p h q", h=2)[:, :, o:512],
                            AF.Exp,
                        )
                        if diag:
                            for hh in range(2):
                                nc.vector.tensor_mul(
                                    aT[:, 512 * hh + o:512 * hh + o + 128],
                                    aT[:, 512 * hh + o:512 * hh + o + 128],
                                    mask_sb[:],
                                )
                        for hh in range(2):
                            nc.tensor.matmul(
                                augs[hh][0:HD + 1, o:512],
                                v_sb[:, c, 2 * j + hh, :],
                                aT[:, 512 * hh + o:512 * hh + 512],
                                start=(c == 0), stop=last,
                            )
                    nb = nbp.tile([128, 512], F32R, tag="nb")
                    for hh in range(2):
                        aug = augs[hh]
                        recip = normp.tile([128, 512], F32, tag="recip")
                        nc.vector.reciprocal(recip[0:1, :], aug[HD:HD + 1, :])
                        bc = normp.tile([64, 512], F32, tag="bc")
                        nc.gpsimd.partition_broadcast(
                            bc[:], recip[0:1, :], channels=64
                        )
                        nc.vector.tensor_mul(
                            nb[64 * hh:64 * (hh + 1), :], aug[0:HD, :], bc[:]
                        )
                    nc.sync.dma_start(
                        out=ci[:, 512 * qb:512 * (qb + 1)], in_=nb[:]
                    )
                co = ccoutp.tile([256, T], F32R, tag="co", name="co")
                if single_core:
                    # timing stand-in for the pairwise AllGather
                    nc.sync.dma_start(out=co[0:128, :], in_=ci[:])
                    nc.sync.dma_start(out=co[128:256, :], in_=ci[:])
                else:
                    nc.gpsimd.collective_compute(
                        "AllGather",
                        mybir.AluOpType.bypass,
                        replica_groups=REPLICA_GROUPS,
                        ins=[ci.opt()],
                        outs=[co.opt()],
                    )
                cc_outs.append(co)

        # ---------------- output projection ----------------
        with tc.tile_pool(name="apf", bufs=2 * NPAIR) as apf, \
             tc.tile_pool(name="ysb", bufs=3) as ysbp:
            att_tiles = []  # (c_in block index, tile)
            for j in range(NPAIR):
                for half in range(2):
                    t_ = apf.tile([128, T], F32R, tag="apf", name="apf")
                    for pc in range(2):
                        nc.sync.dma_start(
                            out=t_[:, 1024 * pc:1024 * (pc + 1)],
                            in_=cc_outs[j][128 * half:128 * (half + 1),
                                           1024 * pc:1024 * (pc + 1)],
                        )
                    att_tiles.append((4 * half + j, t_))
            for tb in range(TB):
                ps = pss.tile([128, 1024], F32, tag="s2", name="psy")
                for idx, (a, t_) in enumerate(att_tiles):
                    nc.tensor.matmul(
                        ps[:, 0:512],
                        t_[:, 128 * tb:128 * (tb + 1)],
                        wp_sb[:, a, :],
                        start=(idx == 0), stop=(idx == len(att_tiles) - 1),
                    )
                ysb = ysbp.tile([128, 512], F32, tag="ysb")
                nc.vector.tensor_copy(ysb[:], ps[:, 0:512])
                nc.sync.dma_start(
                    out=y[128 * tb:128 * (tb + 1), :], in_=ysb[:]
                )



_NC_CACHE = None


def _get_nc():
    global _NC_CACHE
    if _NC_CACHE is None:
        _NC_CACHE = build_nc()
    return _NC_CACHE


def _mask_np():
    # mask[kv', q'] = 1 where q' >= kv' (within-chunk causal triangle)
    kv = np.arange(128)[:, None]
    q = np.arange(128)[None, :]
    return (q >= kv).astype(np.float32)


def shard_inputs(x, w_qkv, w_proj):
    x = np.asarray(x, dtype=np.float32)
    w_qkv = np.asarray(w_qkv, dtype=np.float32)
    w_proj = np.asarray(w_proj, dtype=np.float32)
    mask = _mask_np()
    in_maps = []
    for core in range(N_CORES):
        pair, rank = divmod(core, 2)
        c0 = HD * HPC * rank  # 0 or 512: this core's head-column offset
        in_maps.append({
            "xt": np.ascontiguousarray(x[pair].T),
            "wq": np.ascontiguousarray(w_qkv[:, c0:c0 + 512]),
            "wk": np.ascontiguousarray(w_qkv[:, C + c0:C + c0 + 512]),
            "wv": np.ascontiguousarray(w_qkv[:, 2 * C + c0:2 * C + c0 + 512]),
            "wp": np.ascontiguousarray(w_proj[:, 512 * rank:512 * rank + 512]),
            "mask": mask,
        })
    return in_maps


def assemble_output(results):
    out = np.empty((B, T, C), dtype=np.float32)
    for core in range(N_CORES):
        pair, rank = divmod(core, 2)
        out[pair][:, 512 * rank:512 * rank + 512] = results[core]["y"]
    return out


# --- cached PJRT runner (same path run_bass_kernel_spmd takes under axon,
# but keeps the jitted executable so repeat calls skip re-tracing) ---
_RUNNER_CACHE = None


def _make_runner(nc):
    import jax
    import numpy as _np
    from jax.sharding import Mesh, PartitionSpec
    from jax.experimental.shard_map import shard_map
    from concourse import bass2jax
    from concourse.bass2jax import _bass_exec_p, install_neuronx_cc_hook

    install_neuronx_cc_hook()
    part_name = (nc.partition_id_tensor.name
                 if nc.partition_id_tensor else None)
    in_names, out_names, out_avals, zero_shapes = [], [], [], []
    for alloc in nc.m.functions[0].allocations:
        if not isinstance(alloc, mybir.MemoryLocationSet):
            continue
        name = alloc.memorylocations[0].name
        if alloc.kind == "ExternalInput":
            if name != part_name:
                in_names.append(name)
        elif alloc.kind == "ExternalOutput":
            out_names.append(name)
            shape = tuple(alloc.tensor_shape)
            dtype = mybir.dt.np(alloc.dtype)
            out_avals.append(jax.core.ShapedArray(shape, dtype))
            zero_shapes.append((shape, dtype))
    n_params = len(in_names)
    n_outs = len(out_names)
    all_in_names = in_names + out_names
    if part_name is not None:
        all_in_names = all_in_names + [part_name]

    def _body(*args):
        operands = list(args)
        if part_name is not None:
            operands.append(bass2jax.partition_id_tensor())
        outs = _bass_exec_p.bind(
            *operands,
            out_avals=tuple(out_avals),
            in_names=tuple(all_in_names),
            out_names=tuple(out_names),
            lowering_input_output_aliases=(),
            sim_require_finite=True,
            sim_require_nnan=True,
            nc=nc,
        )
        return tuple(outs)

    devices = jax.devices()[:N_CORES]
    mesh = Mesh(_np.asarray(devices), ("core",))
    in_specs = (PartitionSpec("core"),) * (n_params + n_outs)
    out_specs = (PartitionSpec("core"),) * n_outs
    donate = tuple(range(n_params, n_params + n_outs))
    sharded = jax.jit(
        shard_map(_body, mesh=mesh, in_specs=in_specs, out_specs=out_specs,
                  check_rep=False),
        donate_argnums=donate, keep_unused=True,
    )

    def run(in_maps):
        concat_in = [
            _np.concatenate([_np.asarray(in_maps[c][nm]) for c in
                             range(N_CORES)], axis=0)
            for nm in in_names
        ]
        concat_zeros = [
            _np.zeros((N_CORES * s[0], *s[1:]), d) for s, d in zero_shapes
        ]
        out_arrs = sharded(*concat_in, *concat_zeros)
        return [
            {nm: _np.asarray(out_arrs[i]).reshape(
                N_CORES, *out_avals[i].shape)[c]
             for i, nm in enumerate(out_names)}
            for c in range(N_CORES)
        ]

    run.sharded = sharded
    run.in_names = in_names
    run.zero_shapes = zero_shapes
    run.mesh = mesh
    return run


def _get_runner():
    global _RUNNER_CACHE
    if _RUNNER_CACHE is None:
        _RUNNER_CACHE = _make_runner(_get_nc())
    return _RUNNER_CACHE


def kernel(x, w_qkv, w_proj):
    in_maps = shard_inputs(x, w_qkv, w_proj)
    try:
        results = _get_runner()(in_maps)
    except Exception:
        res = run_bass_kernel_spmd(_get_nc(), in_maps, list(range(N_CORES)))
        results = res.results
    return assemble_output(results)



# revision 3
# speedup vs baseline: 1.1692x; 1.1692x over previous
"""Causal self-attention (B=4, T=2048, C=1024, NH=16) on 8 TRN2 NeuronCores.

Sharding (tensor-parallel heads x data-parallel batch), same as the
previous revision:
  - 4 core-pairs: pair p = cores (2p, 2p+1) handles batch b = p.
  - Within a pair, rank 0 computes heads 0-7, rank 1 heads 8-15.
  - After attention each core holds attnT [512, T] (d-major). A pairwise
    AllGather exchanges these; each core computes a 512-column half of the
    output projection. Host concatenates the halves.

Device algorithm (rewritten for PE-row economy + overlap):
  * Everything off-PSUM is float16 (PSUM accumulation stays fp32), which
    halves DMA bytes and lifts the fp32r free-dim>=256 restriction.
  * a@V runs q-major: out[q 128, d 65] = aT_chunk.T @ v_aug, M=128 instead
    of the old d-major M=65 — half the PE rows for the same math. The
    65th column of v_aug is ones, so column 64 accumulates the softmax
    denominator per q row; normalization is then a per-partition
    reciprocal + tensor_scalar multiply (no partition broadcast).
  * The normalized [128 q, 128 (2 heads x 64 d)] tile is flipped back to
    d-major with an SBUF->SBUF XBAR DMA transpose (14ns/tile, off the PE).
  * exp runs on ScalarE at 2x the PE's per-row cost, so raw attention is
    ACT-paced. All GEMM work (v, q/k, and the pairs-0..2 part of the
    projection) is emitted through a filler queue interleaved into the
    attention chunk loop, keeping the PE busy while ACT catches up.
  * q stays in SBUF (no DRAM spill), PSUM uses exactly 8 banks:
    2 (qkv/proj accum) + 4 (scores double-buffered) + 2 (a@V accum).
"""

import numpy as np
from collections import deque

import concourse.bass as bass
import concourse.mybir as mybir
import concourse.tile as tile
from concourse import bacc
from concourse.bass_utils import run_bass_kernel_spmd

B, T, C = 4, 2048, 1024
NH, HD = 16, 64
N_CORES = 8
HPC = NH // 2          # heads per core
NPAIR = HPC // 2       # head-pairs per core
TB = T // 128          # 128-row t blocks
QBS = T // 512         # 512-wide q blocks
KC = C // 128          # 128-deep contraction chunks for qkv/proj
SCALE = float(1.0 / np.sqrt(HD))

F32 = mybir.dt.float32
F16 = mybir.dt.float16
AF = mybir.ActivationFunctionType
REPLICA_GROUPS = [[0, 1], [2, 3], [4, 5], [6, 7]]

# cost-model constants used only to pace filler emission (ns)
PE_ROW = 0.4167
ACT_ROW = 0.8333
ACT_FIX = 143.0


def build_nc(reps=1, single_core=False):
    nc = bacc.Bacc(
        "TRN2", target_bir_lowering=False, debug=False,
        num_devices=(1 if single_core else N_CORES),
    )

    xt = nc.dram_tensor("xt", [C, T], F16, kind="ExternalInput")
    wq = nc.dram_tensor("wq", [C, 512], F16, kind="ExternalInput")
    wk = nc.dram_tensor("wk", [C, 512], F16, kind="ExternalInput")
    wv = nc.dram_tensor("wv", [C, 512], F16, kind="ExternalInput")
    wp = nc.dram_tensor("wp", [C, 512], F16, kind="ExternalInput")
    mask = nc.dram_tensor("mask", [128, 2, 128], F16, kind="ExternalInput")
    y = nc.dram_tensor("y", [T, 512], F16, kind="ExternalOutput")

    with tile.TileContext(nc) as tc:
        for _rep in range(reps):
            _emit_one(nc, tc, xt, wq, wk, wv, wp, mask, y, single_core)

    nc.compile()
    return nc


def _emit_one(nc, tc, xt, wq, wk, wv, wp, mask, y, single_core):
    with tc.tile_pool(name="persist", bufs=1) as persist, \
         tc.tile_pool(name="xtp", bufs=1) as xtp, \
         tc.tile_pool(name="wvp", bufs=1) as wvp, \
         tc.tile_pool(name="wqk", bufs=2) as wqkp, \
         tc.tile_pool(name="qp", bufs=2) as qp, \
         tc.tile_pool(name="atp", bufs=4) as atp, \
         tc.tile_pool(name="rcpp", bufs=2) as rcpp, \
         tc.tile_pool(name="nbp", bufs=3) as nbp, \
         tc.tile_pool(name="cisp", bufs=2) as cisp, \
         tc.tile_pool(name="apfp", bufs=8) as apfp, \
         tc.tile_pool(name="partp", bufs=16) as partp, \
         tc.tile_pool(name="ysbp", bufs=2) as ysbp, \
         tc.tile_pool(name="ccin", bufs=2, space="DRAM") as ccinp, \
         tc.tile_pool(name="ccout", bufs=NPAIR, space="DRAM") as ccoutp, \
         tc.tile_pool(name="psq", bufs=2, space="PSUM") as psq, \
         tc.tile_pool(name="ps2", bufs=2, space="PSUM") as ps2, \
         tc.tile_pool(name="paug", bufs=1, space="PSUM") as paug:

        kT_sb = persist.tile([128, NPAIR, T], F16)
        v_sb = persist.tile([128, TB, HPC, HD + 1], F16)
        wp_sb = persist.tile([128, KC, 512], F16)
        mask_sb = persist.tile([128, 2, 128], F16)
        xt_sb = xtp.tile([128, KC, T], F16)
        wv_sb = wvp.tile([128, KC, 512], F16)

        xt_r = xt[:].rearrange("(a p) t -> p a t", p=128)
        wv_r = wv[:].rearrange("(a p) n -> p a n", p=128)
        wq_r = wq[:].rearrange("(a p) n -> p a n", p=128)
        wk_r = wk[:].rearrange("(a p) n -> p a n", p=128)
        wpr = wp[:].rearrange("(a p) n -> p a n", p=128)

        # upfront DMAs; first compute needs xt chunk 0 + wv, so they go first
        nc.sync.dma_start(out=xt_sb[:, 0, :], in_=xt_r[:, 0, :])
        nc.sync.dma_start(out=wv_sb[:], in_=wv_r[:])
        wq_tiles, wk_tiles = {}, {}

        def fetch_wqk(j):
            wq_tiles[j] = wqkp.tile([128, KC, 128], F16, tag="wq", name=f"wq{j}")
            wk_tiles[j] = wqkp.tile([128, KC, 128], F16, tag="wk", name=f"wk{j}")
            nc.sync.dma_start(
                out=wq_tiles[j][:], in_=wq_r[:, :, 128 * j:128 * (j + 1)])
            nc.sync.dma_start(
                out=wk_tiles[j][:], in_=wk_r[:, :, 128 * j:128 * (j + 1)])

        fetch_wqk(0)
        nc.sync.dma_start(out=mask_sb[:], in_=mask[:])
        for a in range(1, KC):
            nc.sync.dma_start(out=xt_sb[:, a, :], in_=xt_r[:, a, :])
        nc.sync.dma_start(out=wp_sb[:], in_=wpr[:])
        nc.vector.memset(v_sb[:, :, :, HD], 1.0)

        q_tiles = {}
        apf_tiles = {}
        part_tiles = {}
        state = {"deficit": 0.0, "co_ready": 0}

        # ---------------- filler units (PE-feeding work) ----------------
        def v_unit(tb):
            def emit():
                ps = psq.tile([128, 512], F32, tag="acc", name="psv")
                for a in range(KC):
                    nc.tensor.matmul(
                        ps[:], xt_sb[:, a, 128 * tb:128 * (tb + 1)],
                        wv_sb[:, a, :], start=(a == 0), stop=(a == KC - 1))
                nc.vector.tensor_copy(
                    v_sb[:, tb, :, 0:HD],
                    ps[:].rearrange("p (h d) -> p h d", h=HPC))
            return ("v", emit, 8 * 512 * PE_ROW)

        def qk_unit(j, which, i):
            def emit():
                if which == "q" and j not in q_tiles:
                    q_tiles[j] = qp.tile([128, T], F16, tag="q",
                                         name=f"q{j}")
                wsb = wq_tiles[j] if which == "q" else wk_tiles[j]
                ps = psq.tile([128, 512], F32, tag="acc", name="psqk")
                for a in range(KC):
                    nc.tensor.matmul(
                        ps[:], wsb[:, a, :],
                        xt_sb[:, a, 512 * i:512 * (i + 1)],
                        start=(a == 0), stop=(a == KC - 1))
                if which == "q":
                    nc.vector.tensor_scalar_mul(
                        q_tiles[j][:, 512 * i:512 * (i + 1)], ps[:], SCALE)
                else:
                    nc.vector.tensor_copy(
                        kT_sb[:, j, 512 * i:512 * (i + 1)], ps[:])
            return ("qk", emit, 8 * 512 * PE_ROW)

        PROJ_EARLY = [0, 1, 2, 4, 5, 6]   # c_att blocks of pairs 0-2
        PROJ_LATE = [3, 7]                # pair 3

        def proj_early_unit(tb):
            def emit():
                ps = psq.tile([128, 512], F32, tag="acc", name="psp")
                for idx, a in enumerate(PROJ_EARLY):
                    nc.tensor.matmul(
                        ps[:], apf_tiles[a][:, 128 * tb:128 * (tb + 1)],
                        wp_sb[:, a, :],
                        start=(idx == 0), stop=(idx == len(PROJ_EARLY) - 1))
                part_tiles[tb] = partp.tile([128, 512], F32, tag="part",
                                            name=f"part{tb}")
                nc.vector.tensor_copy(part_tiles[tb][:], ps[:])
            return ("proj", emit, len(PROJ_EARLY) * 512 * PE_ROW)

        fillers = deque()
        markers = {}
        for i in range(QBS):
            for tb in range(4 * i, 4 * (i + 1)):
                fillers.append(v_unit(tb))
            fillers.append(qk_unit(0, "q", i))
            fillers.append(qk_unit(0, "k", i))
            markers[(0, i)] = len(fillers)
        for j in range(1, NPAIR):
            for i in range(QBS):
                fillers.append(qk_unit(j, "q", i))
                fillers.append(qk_unit(j, "k", i))
                markers[(j, i)] = len(fillers)
        for tb in range(TB):
            fillers.append(proj_early_unit(tb))
        n_popped = [0]

        def pop_one():
            kind, emit, pe_ns = fillers.popleft()
            emit()
            n_popped[0] += 1
            state["deficit"] -= pe_ns

        def pop_for_deficit():
            while fillers and state["deficit"] > 900:
                if fillers[0][0] == "proj" and state["co_ready"] < 3:
                    return
                pop_one()

        def force_through(marker):
            while n_popped[0] < marker:
                pop_one()

        # ---------------- attention ----------------
        def att_qb(j, qb):
            aug = paug.tile([128, 2, 4, 128], F32, tag="aug", name="aug")
            nchunks = 4 * (qb + 1)
            for c in range(nchunks):
                diag = c >= 4 * qb
                o = 128 * (c - 4 * qb) if diag else 0
                s2 = ps2.tile([128, 2, 512], F32, tag="s2", name="s2")
                for hh in range(2):
                    nc.tensor.matmul(
                        s2[:, hh, o:512],
                        kT_sb[64 * hh:64 * hh + 64, j, 128 * c:128 * (c + 1)],
                        q_tiles[j][64 * hh:64 * hh + 64,
                                   512 * qb + o:512 * (qb + 1)],
                        start=True, stop=True)
                aT = atp.tile([128, 2, 512], F16, tag="aT", name="aT")
                nc.scalar.activation(aT[:, :, o:512], s2[:, :, o:512], AF.Exp)
                if diag:
                    nc.vector.tensor_mul(
                        aT[:, :, o:o + 128], aT[:, :, o:o + 128], mask_sb[:])
                n_mm = 0
                for s in range(max(0, c - 4 * qb), 4):
                    for hh in range(2):
                        nc.tensor.matmul(
                            aug[:, hh, s, 0:HD + 1],
                            aT[:, hh, 128 * s:128 * (s + 1)],
                            v_sb[:, c, 2 * j + hh, :],
                            start=(c == 0), stop=(c == 4 * qb + s))
                        n_mm += 1
                rows = 2 * (512 - o)
                state["deficit"] += (rows * ACT_ROW + ACT_FIX) \
                    - (rows * PE_ROW + n_mm * 65 * PE_ROW)
                if diag:
                    s0 = c - 4 * qb
                    rcp = rcpp.tile([128, 2], F32, tag="rcp", name="rcp")
                    nc.vector.reciprocal(rcp[:], aug[:, :, s0, HD])
                    nb = nbp.tile([128, 2, HD], F16, tag="nb", name="nb")
                    for hh in range(2):
                        nc.vector.tensor_scalar_mul(
                            nb[:, hh, :], aug[:, hh, s0, 0:HD],
                            rcp[:, hh:hh + 1])
                    nc.sync.dma_start_transpose(
                        ci_sb[:, 512 * qb + 128 * s0:512 * qb + 128 * (s0 + 1)],
                        nb[:])
                pop_for_deficit()

        cc_outs = []
        for j in range(NPAIR):
            ci_sb = cisp.tile([128, T], F16, tag="ci", name=f"ci{j}")
            if j + 1 < NPAIR:
                fetch_wqk(j + 1)
            for qb in range(QBS):
                force_through(markers[(j, qb)])
                att_qb(j, qb)
            ci_d = ccinp.tile([128, T], F16, tag="cid", name=f"cid{j}")
            nc.sync.dma_start(out=ci_d[:], in_=ci_sb[:])
            co = ccoutp.tile([256, T], F16, tag="co", name=f"co{j}")
            if single_core:
                # timing stand-in for the pairwise AllGather
                nc.sync.dma_start(out=co[0:128, :], in_=ci_d[:])
                nc.sync.dma_start(out=co[128:256, :], in_=ci_d[:])
            else:
                nc.gpsimd.collective_compute(
                    "AllGather",
                    mybir.AluOpType.bypass,
                    replica_groups=REPLICA_GROUPS,
                    ins=[ci_d.opt()],
                    outs=[co.opt()],
                )
            cc_outs.append(co)
            for half in range(2):
                t_ = apfp.tile([128, T], F16, tag="apf", name=f"apf{j}{half}")
                nc.sync.dma_start(
                    out=t_[:], in_=co[128 * half:128 * (half + 1), :])
                apf_tiles[4 * half + j] = t_
            state["co_ready"] = j + 1

        while fillers:
            pop_one()

        # ---------------- output projection (pair-3 part + combine) -------
        y_r = y[:].rearrange("(g a p) n -> p g a n", a=4, p=128)
        ysb = None
        for tb in range(TB):
            if tb % 4 == 0:
                ysb = ysbp.tile([128, 4, 512], F16, tag="ysb", name="ysb")
            ps = psq.tile([128, 512], F32, tag="acc", name="psy")
            for idx, a in enumerate(PROJ_LATE):
                nc.tensor.matmul(
                    ps[:], apf_tiles[a][:, 128 * tb:128 * (tb + 1)],
                    wp_sb[:, a, :],
                    start=(idx == 0), stop=(idx == len(PROJ_LATE) - 1))
            nc.vector.tensor_add(ysb[:, tb % 4, :], part_tiles[tb][:], ps[:])
            if tb % 4 == 3:
                nc.sync.dma_start(out=y_r[:, tb // 4, :, :], in_=ysb[:])


_NC_CACHE = None


def _get_nc():
    global _NC_CACHE
    if _NC_CACHE is None:
        _NC_CACHE = build_nc()
    return _NC_CACHE


def _mask_np():
    # mask[kv', hh, q'] = 1 where q' >= kv' (within-chunk causal triangle),
    # duplicated over the two heads packed per score tile
    kv = np.arange(128)[:, None]
    q = np.arange(128)[None, :]
    tri = (q >= kv).astype(np.float16)
    return np.ascontiguousarray(
        np.broadcast_to(tri[:, None, :], (128, 2, 128)))


def shard_inputs(x, w_qkv, w_proj):
    x = np.asarray(x, dtype=np.float16)
    w_qkv = np.asarray(w_qkv, dtype=np.float16)
    w_proj = np.asarray(w_proj, dtype=np.float16)
    mask = _mask_np()
    in_maps = []
    for core in range(N_CORES):
        pair, rank = divmod(core, 2)
        c0 = HD * HPC * rank  # 0 or 512: this core's head-column offset
        in_maps.append({
            "xt": np.ascontiguousarray(x[pair].T),
            "wq": np.ascontiguousarray(w_qkv[:, c0:c0 + 512]),
            "wk": np.ascontiguousarray(w_qkv[:, C + c0:C + c0 + 512]),
            "wv": np.ascontiguousarray(w_qkv[:, 2 * C + c0:2 * C + c0 + 512]),
            "wp": np.ascontiguousarray(w_proj[:, 512 * rank:512 * rank + 512]),
            "mask": mask,
        })
    return in_maps


def assemble_output(results):
    out = np.empty((B, T, C), dtype=np.float32)
    for core in range(N_CORES):
        pair, rank = divmod(core, 2)
        out[pair][:, 512 * rank:512 * rank + 512] = \
            results[core]["y"].astype(np.float32)
    return out


# --- cached PJRT runner (same path run_bass_kernel_spmd takes under axon,
# but keeps the jitted executable so repeat calls skip re-tracing) ---
_RUNNER_CACHE = None


def _make_runner(nc):
    import jax
    import numpy as _np
    from jax.sharding import Mesh, PartitionSpec
    from jax.experimental.shard_map import shard_map
    from concourse import bass2jax
    from concourse.bass2jax import _bass_exec_p, install_neuronx_cc_hook

    install_neuronx_cc_hook()
    part_name = (nc.partition_id_tensor.name
                 if nc.partition_id_tensor else None)
    in_names, out_names, out_avals, zero_shapes = [], [], [], []
    for alloc in nc.m.functions[0].allocations:
        if not isinstance(alloc, mybir.MemoryLocationSet):
            continue
        name = alloc.memorylocations[0].name
        if alloc.kind == "ExternalInput":
            if name != part_name:
                in_names.append(name)
        elif alloc.kind == "ExternalOutput":
            out_names.append(name)
            shape = tuple(alloc.tensor_shape)
            dtype = mybir.dt.np(alloc.dtype)
            out_avals.append(jax.core.ShapedArray(shape, dtype))
            zero_shapes.append((shape, dtype))
    n_params = len(in_names)
    n_outs = len(out_names)
    all_in_names = in_names + out_names
    if part_name is not None:
        all_in_names = all_in_names + [part_name]

    def _body(*args):
        operands = list(args)
        if part_name is not None:
            operands.append(bass2jax.partition_id_tensor())
        outs = _bass_exec_p.bind(
            *operands,
            out_avals=tuple(out_avals),
            in_names=tuple(all_in_names),
            out_names=tuple(out_names),
            lowering_input_output_aliases=(),
            sim_require_finite=True,
            sim_require_nnan=True,
            nc=nc,
        )
        return tuple(outs)

    devices = jax.devices()[:N_CORES]
    mesh = Mesh(_np.asarray(devices), ("core",))
    in_specs = (PartitionSpec("core"),) * (n_params + n_outs)
    out_specs = (PartitionSpec("core"),) * n_outs
    donate = tuple(range(n_params, n_params + n_outs))
    sharded = jax.jit(
        shard_map(_body, mesh=mesh, in_specs=in_specs, out_specs=out_specs,
                  check_rep=False),
        donate_argnums=donate, keep_unused=True,
    )

    def run(in_maps):
        concat_in = [
            _np.concatenate([_np.asarray(in_maps[c][nm]) for c in
                             range(N_CORES)], axis=0)
            for nm in in_names
        ]
        concat_zeros = [
            _np.zeros((N_CORES * s[0], *s[1:]), d) for s, d in zero_shapes
        ]
        out_arrs = sharded(*concat_in, *concat_zeros)
        return [
            {nm: _np.asarray(out_arrs[i]).reshape(
                N_CORES, *out_avals[i].shape)[c]
             for i, nm in enumerate(out_names)}
            for c in range(N_CORES)
        ]

    run.sharded = sharded
    run.in_names = in_names
    run.zero_shapes = zero_shapes
    run.mesh = mesh
    return run


def _get_runner():
    global _RUNNER_CACHE
    if _RUNNER_CACHE is None:
        _RUNNER_CACHE = _make_runner(_get_nc())
    return _RUNNER_CACHE


def kernel(x, w_qkv, w_proj):
    in_maps = shard_inputs(x, w_qkv, w_proj)
    try:
        results = _get_runner()(in_maps)
    except Exception:
        res = run_bass_kernel_spmd(_get_nc(), in_maps, list(range(N_CORES)))
        results = res.results
    return assemble_output(results)
